# revision 70
# baseline (speedup 1.0000x reference)
"""Trainium2 Bass kernel: ConvTranspose3d(32->64,k3,s2,p1) + 0.5x + MaxPool3d(2) +
global-avg-pool + clamp(0,1), data-parallel over batch on 8 NeuronCores.

Math: a stride-2 transposed conv splits into 8 parity classes (even/odd output
index per spatial axis); each 2x2x2 maxpool window holds exactly one output of
each class, so maxpool == elementwise max over the 8 class sub-convolutions.

Conv: fp8(e4m3) DoubleRow matmuls contract K=256 per pass = 4 partition blocks
((dh,dw) shifted copies of x) x 32 c_in x 2 ko k-tiles (the dd shift, a d-axis
stride in the rhs AP).  The full {0,1}^3 shift cube is addressable in one
matmul, so each class's complete tap set fits a single pass: 4 passes x
(2 classes per 128-partition PSUM output) cover all 8 classes at 0.5
cycles/row.  Weights are pre-scaled by 128 into fp8 range; the final mean
scale divides it back out.

Consume (max over 8 classes + mean): PSUM is split into four 2-bank tiles per
pair (chunk x {passes 0-1, passes 2-3}) so each tile is freed by a single
~1us op and the matmul/evac/max pipeline stays decoupled.  ACT evacuates the
pass-0/1 tiles; DVE maxes them against the pass-2/3 tiles directly in PSUM
(1x) on mixed pairs, while heavy pairs ACT-evacuate those too and the max
runs all-SBUF at 2x -- the heavy fraction balances ACT vs DVE (both land
near 41-44us busy, the wall floor of this split since Pool has no tensor
max and DMA CCE only supports add).  Cross-half folds batch two pairs via
DMA partition-stacking so the fold max + fused sum-accumulate use all 128
partitions (pair j's lo half is DMA'd over pair i's hi half once the hi
stack has copied it out); the final pair folds per chunk, and the high-half
running total is reduced + DMA'd down early, hiding both behind pair 14.
"""

import numpy as np

import concourse.bass as bass
import concourse.bacc as bacc
import concourse.mybir as mybir
from concourse.tile import TileContext
from concourse.bass_utils import run_bass_kernel_spmd
from concourse.alu_op_type import AluOpType

# Problem constants (hardcoded per contract)
N_BATCH = 8
IN_C, OUT_C = 32, 64
D, H, W = 16, 32, 32
JD, JH, JW = 15, 31, 31          # pooled output grid
NPOS = JD * JH * JW              # 14415
SCALE = 0.5
WS = 128.0                       # fp8 weight pre-scale
FREE = D * H * W                 # 16384 flat free size per c_in
DSTR, HSTR = H * W, W            # flat strides

BLOCKS = [(0, 0), (0, 1), (1, 0), (1, 1)]          # (dh, dw) per 32-row block
BLOCK_OFF = [dh * HSTR + dw for (dh, dw) in BLOCKS]

CHUNKS = [(jd, h0, hcnt) for jd in range(JD) for (h0, hcnt) in ((0, 16), (16, 15))]
CN = [hcnt * JW for (_, _, hcnt) in CHUNKS]
PAIRW = CN[0] + CN[1]   # 961 cols per pair tile
NPAIR = len(CHUNKS) // 2
NGROUP = (NPAIR + 1) // 2
NPASS = 4

# Classes (pd, ph, pw); pass p computes CLS[p] as PSUM partition halves.
CLS = [
    ((0, 0, 0), (1, 1, 1)),
    ((0, 0, 1), (1, 1, 0)),
    ((0, 1, 0), (1, 0, 1)),
    ((0, 1, 1), (1, 0, 0)),
]

# Pairs whose pass-2/3 tiles are ACT-evacuated too, making the first-level
# max an all-SBUF bf16 2x DVE op; the heavy fraction balances ACT vs DVE and
# the placement was tuned against the cost model (pair 14 heavy shortens the
# tail's DVE backlog).
HEAVY_PAIRS = {3, 5, 7, 10, 12, 14}
SEMI_HEAVY = {(8, 1)}   # single extra chunk to fine-balance ACT vs DVE


def build_wstack(w: np.ndarray) -> np.ndarray:
    """Stack torch-layout ConvTranspose3d weights (in,out,kd,kh,kw) into the
    4 DoubleRow lhsT blocks, one [128, 4*2*128] array: rows = 32*block + c_in;
    cols = 256*pass + 128*ko + 64*half + c_out.  Unused slots stay 0."""
    wstk = np.zeros((128, NPASS * 2 * 128), np.float32)
    for p in range(NPASS):
        for half, (pd, ph, pw) in enumerate(CLS[p]):
            for ko in range(2):
                if pd == 0 and ko == 1:
                    continue
                kd = 1 if pd == 0 else 2 - 2 * ko
                for bidx, (dh, dw) in enumerate(BLOCKS):
                    if dh > ph or dw > pw:
                        continue
                    kh = 1 if ph == 0 else 2 - 2 * dh
                    kw = 1 if pw == 0 else 2 - 2 * dw
                    col = p * 256 + ko * 128 + half * 64
                    wstk[32 * bidx: 32 * bidx + 32, col: col + OUT_C] = (
                        w[:, :, kd, kh, kw] * WS
                    )
    return wstk


def build_nc() -> bass.Bass:
    nc = bacc.Bacc()
    f32 = mybir.dt.float32
    bf16 = mybir.dt.bfloat16
    fp8 = mybir.dt.float8e4

    x_d = nc.declare_dram_parameter("x", [IN_C, FREE], fp8, isOutput=False)
    w_d = nc.declare_dram_parameter("wstk", [128, NPASS * 256], fp8, isOutput=False)
    b_d = nc.declare_dram_parameter("bvec", [OUT_C, 1], f32, isOutput=False)
    o_d = nc.declare_dram_parameter("out", [OUT_C, 1], f32, isOutput=True)

    with TileContext(nc) as tc:
        with (
            tc.tile_pool(name="xp", bufs=1) as xp,
            tc.tile_pool(name="wp", bufs=1) as wp,
            tc.tile_pool(name="ps", bufs=4, space="PSUM") as ps,
            tc.tile_pool(name="mp", bufs=6) as mp,
            tc.tile_pool(name="fp", bufs=2) as fpool,
            tc.tile_pool(name="ap", bufs=1) as ap,
        ):
            # Trigger the ACT table load at t=0 so it overlaps the x DMA.
            warm = ap.tile([1, 1], bf16, tag="warm")
            nc.gpsimd.memset(warm[:, :], 0.0)
            nc.scalar.copy(warm[:, :], warm[:, :])

            wt = wp.tile([128, NPASS * 256], fp8, tag="wt")
            bv = wp.tile([OUT_C, 1], f32, tag="bv")

            xbuf = xp.tile([128, FREE], fp8, tag="x")
            # Shifted x copies, two (dw) blocks per DMA: the source AP's dw
            # dim overlaps the position dim (strides 1 and 1), reading
            # x[c, pos + 32*dh + dw] into partition 64*dh + 32*dw + c.
            # Priority slab (d-rows 0-1) first; the last stage stops at
            # FREE-33 so the +33 shift never overruns (cols past 16350 are
            # never read).
            from bass_rust import AP as RawAP
            SLAB = 2 * DSTR
            HALF = FREE // 2
            stages = [0, SLAB, HALF, FREE - 33]
            xsrc = x_d[:, :].tensor
            for si, (a, b) in enumerate(zip(stages, stages[1:])):
                for dh in range(2):
                    src = RawAP(
                        xsrc, a + HSTR * dh,
                        [[1, 2], [FREE, IN_C], [1, b - a]],
                    )
                    eng = nc.gpsimd if (si == 0 and dh == 1) else nc.sync
                    eng.dma_start(
                        out=xbuf[64 * dh: 64 * dh + 64, a:b], in_=src
                    )
                if si == 0:
                    nc.sync.dma_start(out=wt[:, :], in_=w_d[:, :])
                    nc.gpsimd.dma_start(out=bv[:, :], in_=b_d[:, :])

            # PE p-state warm-up: dummy matmuls on a zeroed tile while the
            # x DMAs are in flight, so the first real matmuls run at full
            # clock.  They write into the first pair's PSUM tile region,
            # which the real matmuls overwrite (start=True).
            wtile = ap.tile([128, 256], bf16, tag="wtile")
            nc.vector.memset(wtile[:, :], 0.0)
            xv = xbuf[:, :].rearrange("p (d h w) -> p d h w", d=D, h=H, w=W)
            wtv = wt[:, :].rearrange("k (p ko m) -> k p ko m", p=NPASS, ko=2)
            # acc column g: partition p<64 = group g's even pair, p>=64 = odd
            acc = ap.tile([128, 12], f32, tag="acc")
            nc.gpsimd.memset(acc[:, :], 0.0)

            def consume_chunk(pi, k, n, o, TC, TM, mm):
                """One chunk's 4 pass-slots (8 classes) -> mm[:, o:o+n]."""
                TCv = TC[:, :].rearrange("p (s n) -> p s n", s=2, n=512)
                TMv = TM[:, :].rearrange("p (s n) -> p s n", s=2, n=512)
                cc = mp.tile([128, 2, 496], bf16, name="cc", tag="cc")
                nc.scalar.copy(cc[:, :, 0:n], TCv[:, :, 0:n])
                m = mp.tile([128, 2, 496], bf16, name="m", tag="m")
                if pi in HEAVY_PAIRS or (pi, k) in SEMI_HEAVY:
                    cm = mp.tile([128, 2, 496], bf16, name="cm", tag="cm")
                    nc.scalar.copy(cm[:, :, 0:n], TMv[:, :, 0:n])
                    nc.vector.tensor_max(m[:, :, 0:n], cc[:, :, 0:n], cm[:, :, 0:n])
                else:
                    nc.vector.tensor_max(m[:, :, 0:n], cc[:, :, 0:n], TMv[:, :, 0:n])
                nc.vector.tensor_max(
                    mm[:, o: o + n], m[:, 0, 0:n], m[:, 1, 0:n]
                )

            def fold_group(gi, mms):
                # tail groups: HWDGE is free there while Pool dispatch serializes
                lo_eng = nc.gpsimd if True else nc.sync
                """Fold 1-2 pairs' cross-half maxes + accumulate.  For a
                2-pair group, pair i's mm tile becomes the lo stack: pair j's
                lo half overwrites i's hi half once the hi stack has it."""
                mmi, mmj = mms
                hi = fpool.tile([128, PAIRW], bf16, name="hi", tag="hi")
                nc.gpsimd.dma_start(out=hi[0:64, :], in_=mmi[OUT_C:128, :])
                nc.sync.dma_start(out=hi[64:128, :], in_=mmj[OUT_C:128, :])
                lo_eng.dma_start(out=mmi[OUT_C:128, :], in_=mmj[0:OUT_C, :])
                mh = fpool.tile([128, PAIRW], bf16, name="mh", tag="mh")
                mg = fpool.tile([128, PAIRW], bf16, name="mg", tag="mg")
                nc.vector.tensor_max(mh[:, :], mmi[:, :], hi[:, :])
                nc.vector.tensor_scalar(
                    mg[:, :], mh[:, :], 1.0, None,
                    op0=AluOpType.mult, op1=AluOpType.add,
                    accum_out=acc[:, gi: gi + 1],
                )

            def fold_single(col, mmi):
                """One pair, low-half accum only (keeps acc[64:] early-final)."""
                hi = fpool.tile([OUT_C, PAIRW], bf16, name="his", tag="his")
                nc.sync.dma_start(out=hi[:, :], in_=mmi[OUT_C:128, :])
                mh = fpool.tile([OUT_C, PAIRW], bf16, name="mhs", tag="mhs")
                mg = fpool.tile([OUT_C, PAIRW], bf16, name="mgs", tag="mgs")
                nc.vector.tensor_max(mh[:, :], mmi[0:OUT_C, :], hi[:, :])
                nc.vector.tensor_scalar(
                    mg[:, :], mh[:, :], 1.0, None,
                    op0=AluOpType.mult, op1=AluOpType.add,
                    accum_out=acc[0:OUT_C, col: col + 1],
                )

            def fold_split(col, mmi):
                """Final pair: per-chunk folds, each firing off its own DMA."""
                for k, (cl, cn) in enumerate(((0, CN[-2]), (CN[-2], CN[-1]))):
                    hic = fpool.tile([OUT_C, 496], bf16,
                                     name=f"hic{k}", tag=f"hic{k}")
                    nc.gpsimd.dma_start(
                        out=hic[:, 0:cn], in_=mmi[OUT_C:128, cl: cl + cn]
                    )
                    mhc = fpool.tile([OUT_C, 496], bf16,
                                     name=f"mhc{k}", tag=f"mhc{k}")
                    mgc = fpool.tile([OUT_C, 496], bf16,
                                     name=f"mgc{k}", tag=f"mgc{k}")
                    nc.vector.tensor_max(
                        mhc[:, 0:cn], mmi[0:OUT_C, cl: cl + cn], hic[:, 0:cn]
                    )
                    nc.vector.tensor_scalar(
                        mgc[:, 0:cn], mhc[:, 0:cn], 1.0, None,
                        op0=AluOpType.mult, op1=AluOpType.add,
                        accum_out=acc[0:OUT_C, col + k: col + k + 1],
                    )

            pending = []
            group = []
            gi = 0
            warm_ps = None
            for pi in range(NPAIR):
                pair = CHUNKS[2 * pi: 2 * pi + 2]
                # 4 two-bank PSUM tiles: (chunk k) x (C = passes 0-1, M = 2-3)
                T = [
                    [
                        ps.tile([128, 2 * 512], f32, name=f"T{k}{r}", tag="bp")
                        for r in range(2)
                    ]
                    for k in range(2)
                ]
                if pi == 0:
                    for _ in range(8):
                        nc.tensor.matmul(
                            T[0][0][:, 0:256], wtile[:, 0:128], wtile[:, :],
                            start=True, stop=True,
                        )
                for k, (jd, h0, hcnt) in enumerate(pair):
                    n = hcnt * JW
                    for p in range(NPASS):
                        nc.tensor.matmul(
                            T[k][p // 2][:, :].rearrange(
                                "p (s n) -> p s n", s=2, n=512
                            )[:, p % 2, 0:n],
                            wtv[:, p, :, :],
                            xv[:, jd: jd + 2, h0: h0 + hcnt, 0:JW],
                            start=True,
                            stop=True,
                            perf_mode=mybir.MatmulPerfMode.DoubleRow,
                        )
                mm = mp.tile([128, PAIRW], bf16, name="mm", tag="mm")
                for k in range(2):
                    n = CN[2 * pi + k]
                    o = 0 if k == 0 else CN[2 * pi]
                    consume_chunk(pi, k, n, o, T[k][0], T[k][1], mm)
                if pi < 14:
                    group.append(mm)
                    if len(group) == 2:
                        pending.append((gi, group))
                        group = []
                        gi += 1
                        while len(pending) > 2:
                            fold_group(*pending.pop(0))
                    if pi == 13:
                        while pending:
                            fold_group(*pending.pop(0))
                        # acc[64:128] is final after group 6: reduce + DMA
                        # the high half down, hiding behind pair 14.
                        toth = ap.tile([OUT_C, 1], f32, tag="toth")
                        nc.vector.reduce_sum(
                            toth[:, :], acc[OUT_C:128, :],
                            axis=mybir.AxisListType.X,
                        )
                        tots = ap.tile([OUT_C, 1], f32, tag="tots")
                        nc.sync.dma_start(out=tots[:, :], in_=toth[:, :])
                else:
                    fold_split(8, mm)

            totl = ap.tile([OUT_C, 1], f32, tag="totl")
            nc.vector.reduce_sum(
                totl[:, :], acc[0:OUT_C, :], axis=mybir.AxisListType.X
            )
            res = ap.tile([OUT_C, 1], f32, tag="res")
            # res = totl * (SCALE/(NPOS*WS)) + (tots*k + 0.5*b) [precomputed].
            nc.vector.scalar_tensor_tensor(
                res[:, :], totl[:, :], SCALE / (NPOS * WS), pre[:, :],
                op0=AluOpType.mult, op1=AluOpType.add,
            )
            out_t = ap.tile([OUT_C, 1], f32, tag="outt")
            nc.vector.tensor_scalar(
                out_t[:, :], res[:, :], 0.0, 1.0,
                op0=AluOpType.max, op1=AluOpType.min,
            )
            nc.sync.dma_start(out=o_d[:, :], in_=out_t[:, :], single_packet=True)

    return nc


_NC_CACHE = None


def _get_nc():
    global _NC_CACHE
    if _NC_CACHE is None:
        _NC_CACHE = build_nc()
        _NC_CACHE.finalize()
    return _NC_CACHE


def run(x, w, b, **spmd_kwargs):
    """Run on 8 cores; returns (output (8,64,1,1,1), BassKernelResults)."""
    import ml_dtypes
    f8 = np.dtype(ml_dtypes.float8_e4m3)
    x = np.ascontiguousarray(x, np.float32)
    wstk = build_wstack(np.asarray(w, np.float32)).astype(f8)
    bvec = (SCALE * np.asarray(b, np.float32)).reshape(OUT_C, 1).copy()
    nc = _get_nc()
    in_maps = [
        {"x": x[i].reshape(IN_C, FREE).astype(f8), "wstk": wstk, "bvec": bvec}
        for i in range(N_BATCH)
    ]
    r = run_bass_kernel_spmd(nc, in_maps, list(range(N_BATCH)), **spmd_kwargs)
    out = np.stack(
        [np.asarray(r.results[i]["out"], np.float32).reshape(OUT_C) for i in range(N_BATCH)]
    )
    return out.reshape(N_BATCH, OUT_C, 1, 1, 1), r


def kernel(x, w, b):
    out, _ = run(x, w, b)
    return out


# revision 71
# speedup vs baseline: 1.0000x; 1.0000x over previous
"""Trainium2 Bass kernel: ConvTranspose3d(32->64,k3,s2,p1) + 0.5x + MaxPool3d(2) +
global-avg-pool + clamp(0,1), data-parallel over batch on 8 NeuronCores.

Math: a stride-2 transposed conv splits into 8 parity classes (even/odd output
index per spatial axis); each 2x2x2 maxpool window holds exactly one output of
each class, so maxpool == elementwise max over the 8 class sub-convolutions.

Conv: fp8(e4m3) DoubleRow matmuls contract K=256 per pass = 4 partition blocks
((dh,dw) shifted copies of x) x 32 c_in x 2 ko k-tiles (the dd shift, a d-axis
stride in the rhs AP).  The full {0,1}^3 shift cube is addressable in one
matmul, so each class's complete tap set fits a single pass: 4 passes x
(2 classes per 128-partition PSUM output) cover all 8 classes at 0.5
cycles/row.  Weights are pre-scaled by 128 into fp8 range; the final mean
scale divides it back out.

Consume (max over 8 classes + mean): PSUM is split into four 2-bank tiles per
pair (chunk x {passes 0-1, passes 2-3}) so each tile is freed by a single
~1us op and the matmul/evac/max pipeline stays decoupled.  ACT evacuates the
pass-0/1 tiles; DVE maxes them against the pass-2/3 tiles directly in PSUM
(1x) on mixed pairs, while heavy pairs ACT-evacuate those too and the max
runs all-SBUF at 2x -- the heavy fraction balances ACT vs DVE (both land
near 41-44us busy, the wall floor of this split since Pool has no tensor
max and DMA CCE only supports add).  Cross-half folds batch two pairs via
DMA partition-stacking so the fold max + fused sum-accumulate use all 128
partitions (pair j's lo half is DMA'd over pair i's hi half once the hi
stack has copied it out); the final pair folds per chunk, and the high-half
running total is reduced + DMA'd down early, hiding both behind pair 14.
"""

import numpy as np

import concourse.bass as bass
import concourse.bacc as bacc
import concourse.mybir as mybir
from concourse.tile import TileContext
from concourse.bass_utils import run_bass_kernel_spmd
from concourse.alu_op_type import AluOpType

# Problem constants (hardcoded per contract)
N_BATCH = 8
IN_C, OUT_C = 32, 64
D, H, W = 16, 32, 32
JD, JH, JW = 15, 31, 31          # pooled output grid
NPOS = JD * JH * JW              # 14415
SCALE = 0.5
WS = 128.0                       # fp8 weight pre-scale
FREE = D * H * W                 # 16384 flat free size per c_in
DSTR, HSTR = H * W, W            # flat strides

BLOCKS = [(0, 0), (0, 1), (1, 0), (1, 1)]          # (dh, dw) per 32-row block
BLOCK_OFF = [dh * HSTR + dw for (dh, dw) in BLOCKS]

CHUNKS = [(jd, h0, hcnt) for jd in range(JD) for (h0, hcnt) in ((0, 16), (16, 15))]
CN = [hcnt * JW for (_, _, hcnt) in CHUNKS]
PAIRW = CN[0] + CN[1]   # 961 cols per pair tile
NPAIR = len(CHUNKS) // 2
NGROUP = (NPAIR + 1) // 2
NPASS = 4

# Classes (pd, ph, pw); pass p computes CLS[p] as PSUM partition halves.
CLS = [
    ((0, 0, 0), (1, 1, 1)),
    ((0, 0, 1), (1, 1, 0)),
    ((0, 1, 0), (1, 0, 1)),
    ((0, 1, 1), (1, 0, 0)),
]

# Pairs whose pass-2/3 tiles are ACT-evacuated too, making the first-level
# max an all-SBUF bf16 2x DVE op; the heavy fraction balances ACT vs DVE and
# the placement was tuned against the cost model (pair 14 heavy shortens the
# tail's DVE backlog).
HEAVY_PAIRS = {3, 5, 7, 10, 12, 14}
SEMI_HEAVY = {(8, 1)}   # single extra chunk to fine-balance ACT vs DVE


def build_wstack(w: np.ndarray) -> np.ndarray:
    """Stack torch-layout ConvTranspose3d weights (in,out,kd,kh,kw) into the
    4 DoubleRow lhsT blocks, one [128, 4*2*128] array: rows = 32*block + c_in;
    cols = 256*pass + 128*ko + 64*half + c_out.  Unused slots stay 0."""
    wstk = np.zeros((128, NPASS * 2 * 128), np.float32)
    for p in range(NPASS):
        for half, (pd, ph, pw) in enumerate(CLS[p]):
            for ko in range(2):
                if pd == 0 and ko == 1:
                    continue
                kd = 1 if pd == 0 else 2 - 2 * ko
                for bidx, (dh, dw) in enumerate(BLOCKS):
                    if dh > ph or dw > pw:
                        continue
                    kh = 1 if ph == 0 else 2 - 2 * dh
                    kw = 1 if pw == 0 else 2 - 2 * dw
                    col = p * 256 + ko * 128 + half * 64
                    wstk[32 * bidx: 32 * bidx + 32, col: col + OUT_C] = (
                        w[:, :, kd, kh, kw] * WS
                    )
    return wstk


def build_nc() -> bass.Bass:
    nc = bacc.Bacc()
    f32 = mybir.dt.float32
    bf16 = mybir.dt.bfloat16
    fp8 = mybir.dt.float8e4

    x_d = nc.declare_dram_parameter("x", [IN_C, FREE], fp8, isOutput=False)
    w_d = nc.declare_dram_parameter("wstk", [128, NPASS * 256], fp8, isOutput=False)
    b_d = nc.declare_dram_parameter("bvec", [OUT_C, 1], f32, isOutput=False)
    o_d = nc.declare_dram_parameter("out", [OUT_C, 1], f32, isOutput=True)

    with TileContext(nc) as tc:
        with (
            tc.tile_pool(name="xp", bufs=1) as xp,
            tc.tile_pool(name="wp", bufs=1) as wp,
            tc.tile_pool(name="ps", bufs=4, space="PSUM") as ps,
            tc.tile_pool(name="mp", bufs=5) as mp,
            tc.tile_pool(name="fp", bufs=2) as fpool,
            tc.tile_pool(name="ap", bufs=1) as ap,
        ):
            # Trigger the ACT table load at t=0 so it overlaps the x DMA.
            warm = ap.tile([1, 1], bf16, tag="warm")
            nc.gpsimd.memset(warm[:, :], 0.0)
            nc.scalar.copy(warm[:, :], warm[:, :])

            wt = wp.tile([128, NPASS * 256], fp8, tag="wt")
            bv = wp.tile([OUT_C, 1], f32, tag="bv")

            xbuf = xp.tile([128, FREE], fp8, tag="x")
            # Shifted x copies, two (dw) blocks per DMA: the source AP's dw
            # dim overlaps the position dim (strides 1 and 1), reading
            # x[c, pos + 32*dh + dw] into partition 64*dh + 32*dw + c.
            # Priority slab (d-rows 0-1) first; the last stage stops at
            # FREE-33 so the +33 shift never overruns (cols past 16350 are
            # never read).
            from bass_rust import AP as RawAP
            SLAB = 2 * DSTR
            HALF = FREE // 2
            stages = [0, SLAB, HALF, FREE - 33]
            xsrc = x_d[:, :].tensor
            for si, (a, b) in enumerate(zip(stages, stages[1:])):
                for dh in range(2):
                    src = RawAP(
                        xsrc, a + HSTR * dh,
                        [[1, 2], [FREE, IN_C], [1, b - a]],
                    )
                    eng = nc.gpsimd if (si == 0 and dh == 1) else nc.sync
                    eng.dma_start(
                        out=xbuf[64 * dh: 64 * dh + 64, a:b], in_=src
                    )
                if si == 0:
                    nc.sync.dma_start(out=wt[:, :], in_=w_d[:, :])
                    nc.gpsimd.dma_start(out=bv[:, :], in_=b_d[:, :])

            # PE p-state warm-up: dummy matmuls on a zeroed tile while the
            # x DMAs are in flight, so the first real matmuls run at full
            # clock.  They write into the first pair's PSUM tile region,
            # which the real matmuls overwrite (start=True).
            wtile = ap.tile([128, 256], bf16, tag="wtile")
            nc.vector.memset(wtile[:, :], 0.0)
            xv = xbuf[:, :].rearrange("p (d h w) -> p d h w", d=D, h=H, w=W)
            wtv = wt[:, :].rearrange("k (p ko m) -> k p ko m", p=NPASS, ko=2)
            # acc column g: partition p<64 = group g's even pair, p>=64 = odd
            acc = ap.tile([128, 12], f32, tag="acc")
            nc.gpsimd.memset(acc[:, :], 0.0)

            def consume_chunk(pi, k, n, o, TC, TM, mm):
                """One chunk's 4 pass-slots (8 classes) -> mm[:, o:o+n]."""
                TCv = TC[:, :].rearrange("p (s n) -> p s n", s=2, n=512)
                TMv = TM[:, :].rearrange("p (s n) -> p s n", s=2, n=512)
                cc = mp.tile([128, 2, 496], bf16, name="cc", tag="cc")
                nc.scalar.copy(cc[:, :, 0:n], TCv[:, :, 0:n])
                m = mp.tile([128, 2, 496], bf16, name="m", tag="m")
                if pi in HEAVY_PAIRS or (pi, k) in SEMI_HEAVY:
                    cm = mp.tile([128, 2, 496], bf16, name="cm", tag="cm")
                    nc.scalar.copy(cm[:, :, 0:n], TMv[:, :, 0:n])
                    nc.vector.tensor_max(m[:, :, 0:n], cc[:, :, 0:n], cm[:, :, 0:n])
                else:
                    nc.vector.tensor_max(m[:, :, 0:n], cc[:, :, 0:n], TMv[:, :, 0:n])
                nc.vector.tensor_max(
                    mm[:, o: o + n], m[:, 0, 0:n], m[:, 1, 0:n]
                )

            def fold_group(gi, mms):
                # tail groups: HWDGE is free there while Pool dispatch serializes
                lo_eng = nc.gpsimd if True else nc.sync
                """Fold 1-2 pairs' cross-half maxes + accumulate.  For a
                2-pair group, pair i's mm tile becomes the lo stack: pair j's
                lo half overwrites i's hi half once the hi stack has it."""
                mmi, mmj = mms
                hi = fpool.tile([128, PAIRW], bf16, name="hi", tag="hi")
                nc.gpsimd.dma_start(out=hi[0:64, :], in_=mmi[OUT_C:128, :])
                nc.sync.dma_start(out=hi[64:128, :], in_=mmj[OUT_C:128, :])
                lo_eng.dma_start(out=mmi[OUT_C:128, :], in_=mmj[0:OUT_C, :])
                mh = fpool.tile([128, PAIRW], bf16, name="mh", tag="mh")
                mg = fpool.tile([128, PAIRW], bf16, name="mg", tag="mg")
                nc.vector.tensor_max(mh[:, :], mmi[:, :], hi[:, :])
                nc.vector.tensor_scalar(
                    mg[:, :], mh[:, :], 1.0, None,
                    op0=AluOpType.mult, op1=AluOpType.add,
                    accum_out=acc[:, gi: gi + 1],
                )

            def fold_single(col, mmi):
                """One pair, low-half accum only (keeps acc[64:] early-final)."""
                hi = fpool.tile([OUT_C, PAIRW], bf16, name="his", tag="his")
                nc.sync.dma_start(out=hi[:, :], in_=mmi[OUT_C:128, :])
                mh = fpool.tile([OUT_C, PAIRW], bf16, name="mhs", tag="mhs")
                mg = fpool.tile([OUT_C, PAIRW], bf16, name="mgs", tag="mgs")
                nc.vector.tensor_max(mh[:, :], mmi[0:OUT_C, :], hi[:, :])
                nc.vector.tensor_scalar(
                    mg[:, :], mh[:, :], 1.0, None,
                    op0=AluOpType.mult, op1=AluOpType.add,
                    accum_out=acc[0:OUT_C, col: col + 1],
                )

            def fold_split(col, mmi):
                """Final pair: per-chunk folds, each firing off its own DMA."""
                for k, (cl, cn) in enumerate(((0, CN[-2]), (CN[-2], CN[-1]))):
                    hic = fpool.tile([OUT_C, 496], bf16,
                                     name=f"hic{k}", tag=f"hic{k}")
                    nc.gpsimd.dma_start(
                        out=hic[:, 0:cn], in_=mmi[OUT_C:128, cl: cl + cn]
                    )
                    mhc = fpool.tile([OUT_C, 496], bf16,
                                     name=f"mhc{k}", tag=f"mhc{k}")
                    mgc = fpool.tile([OUT_C, 496], bf16,
                                     name=f"mgc{k}", tag=f"mgc{k}")
                    nc.vector.tensor_max(
                        mhc[:, 0:cn], mmi[0:OUT_C, cl: cl + cn], hic[:, 0:cn]
                    )
                    nc.vector.tensor_scalar(
                        mgc[:, 0:cn], mhc[:, 0:cn], 1.0, None,
                        op0=AluOpType.mult, op1=AluOpType.add,
                        accum_out=acc[0:OUT_C, col + k: col + k + 1],
                    )

            pending = []
            group = []
            gi = 0
            warm_ps = None
            for pi in range(NPAIR):
                pair = CHUNKS[2 * pi: 2 * pi + 2]
                # 4 two-bank PSUM tiles: (chunk k) x (C = passes 0-1, M = 2-3)
                T = [
                    [
                        ps.tile([128, 2 * 512], f32, name=f"T{k}{r}", tag="bp")
                        for r in range(2)
                    ]
                    for k in range(2)
                ]
                if pi == 0:
                    for _ in range(8):
                        nc.tensor.matmul(
                            T[0][0][:, 0:256], wtile[:, 0:128], wtile[:, :],
                            start=True, stop=True,
                        )
                for k, (jd, h0, hcnt) in enumerate(pair):
                    n = hcnt * JW
                    for p in range(NPASS):
                        nc.tensor.matmul(
                            T[k][p // 2][:, :].rearrange(
                                "p (s n) -> p s n", s=2, n=512
                            )[:, p % 2, 0:n],
                            wtv[:, p, :, :],
                            xv[:, jd: jd + 2, h0: h0 + hcnt, 0:JW],
                            start=True,
                            stop=True,
                            perf_mode=mybir.MatmulPerfMode.DoubleRow,
                        )
                mm = mp.tile([128, PAIRW], bf16, name="mm", tag="mm")
                for k in range(2):
                    n = CN[2 * pi + k]
                    o = 0 if k == 0 else CN[2 * pi]
                    consume_chunk(pi, k, n, o, T[k][0], T[k][1], mm)
                if pi < 14:
                    group.append(mm)
                    if len(group) == 2:
                        pending.append((gi, group))
                        group = []
                        gi += 1
                        while len(pending) > 2:
                            fold_group(*pending.pop(0))
                    if pi == 13:
                        while pending:
                            fold_group(*pending.pop(0))
                        # acc[64:128] is final after group 6: reduce + DMA
                        # the high half down, hiding behind pair 14.
                        toth = ap.tile([OUT_C, 1], f32, tag="toth")
                        nc.vector.reduce_sum(
                            toth[:, :], acc[OUT_C:128, :],
                            axis=mybir.AxisListType.X,
                        )
                        tots = ap.tile([OUT_C, 1], f32, tag="tots")
                        nc.sync.dma_start(out=tots[:, :], in_=toth[:, :])
                else:
                    fold_split(8, mm)

            totl = ap.tile([OUT_C, 1], f32, tag="totl")
            nc.vector.reduce_sum(
                totl[:, :], acc[0:OUT_C, :], axis=mybir.AxisListType.X
            )
            res = ap.tile([OUT_C, 1], f32, tag="res")
            # res = totl * (SCALE/(NPOS*WS)) + (tots*k + 0.5*b) [precomputed].
            nc.vector.scalar_tensor_tensor(
                res[:, :], totl[:, :], SCALE / (NPOS * WS), pre[:, :],
                op0=AluOpType.mult, op1=AluOpType.add,
            )
            out_t = ap.tile([OUT_C, 1], f32, tag="outt")
            nc.vector.tensor_scalar(
                out_t[:, :], res[:, :], 0.0, 1.0,
                op0=AluOpType.max, op1=AluOpType.min,
            )
            nc.sync.dma_start(out=o_d[:, :], in_=out_t[:, :], single_packet=True)

    return nc


_NC_CACHE = None


def _get_nc():
    global _NC_CACHE
    if _NC_CACHE is None:
        _NC_CACHE = build_nc()
        _NC_CACHE.finalize()
    return _NC_CACHE


def run(x, w, b, **spmd_kwargs):
    """Run on 8 cores; returns (output (8,64,1,1,1), BassKernelResults)."""
    import ml_dtypes
    f8 = np.dtype(ml_dtypes.float8_e4m3)
    x = np.ascontiguousarray(x, np.float32)
    wstk = build_wstack(np.asarray(w, np.float32)).astype(f8)
    bvec = (SCALE * np.asarray(b, np.float32)).reshape(OUT_C, 1).copy()
    nc = _get_nc()
    in_maps = [
        {"x": x[i].reshape(IN_C, FREE).astype(f8), "wstk": wstk, "bvec": bvec}
        for i in range(N_BATCH)
    ]
    r = run_bass_kernel_spmd(nc, in_maps, list(range(N_BATCH)), **spmd_kwargs)
    out = np.stack(
        [np.asarray(r.results[i]["out"], np.float32).reshape(OUT_C) for i in range(N_BATCH)]
    )
    return out.reshape(N_BATCH, OUT_C, 1, 1, 1), r


def kernel(x, w, b):
    out, _ = run(x, w, b)
    return out
